# revision 38
# baseline (speedup 1.0000x reference)
"""ALayer kernel for 8 TRN2 NeuronCores — pure data parallel over batch.

Per-core shard: 4 images of [256, 56, 56].
  h  = relu(conv3x3(x_in, w1))      # 256 -> 16 ch
  A  = sigmoid(conv3x3(h, w2))      # 16 -> 1 ch
  out = x_out * box3x3(A)           # broadcast over 256 ch

v16 design — PE-slot-minimal stream against the DMA roofline
(HBM ~16.3MB/core ~= 45.5us; PE stream sized to match at the realistic
sustained 2.0 GHz P0 clock):
  conv1: v6's column-tiled rounds (4 concurrent 32-col strips, fp8,
         M=16, N=392; 36 rounds/image).  Relu evacs write the h plane
         = hcol[0:16] (2 ACT + 2 DVE per supergroup).
  hcol:  K-folded im2col for conv2: 4 chained SBUF->SBUF copies build
         taps 1..7 (hcol[16t+m, i] = plane[m, i + 58dy + dx]); tap 8 is
         read straight off the plane with a (+2,+2) AP.  The copies ride
         the SP HWDGE ring, emitted JUST-IN-TIME between load DMAs, so
         descriptor gen is RTL-fast and the ring is idle when they fire
         (chain ~2.5us; the SWDGE/Q7 path measured 5-7us and starved
         under load floods).
  conv2: per supergroup ONE K=128 col-tiled round (8 folded taps) plus
         one zero-padded-K round (tap 8) -> 4 PE rounds/image (was 18).
  a9:    v6 scatter construction (6 scatters + 2 row-shifts) on Q7,
         which now carries nothing else.
  box:   7 blocks of 8 rows: K=9-in-128 ones matmul (zero-padded K so
         the HAM activity monitor sees a busy array) -> psum [128,8,56];
         evacs to bf16 `ab` split ACT/DVE; muls are bf16 tensor_tensor
         (DVE 2x) in <=1us chunks that never head-of-line-block the
         latency-critical relu evacs in the DVE FIFO.
  Schedule: one dense PE FIFO, software-pipelined: block(i) = conv1(i)
         rounds with conv2(i-1) at slots ~25-32 (hcol chain done) and
         box(i-2) at the block end (a9 chain done); per-engine FIFOs
         are kept readiness-monotone.  Loads are emitted JIT so the SP
         ring order matches need and SWDGE a9 scatters get idle DMA
         windows.  Short fp8 warm matmuls cover dep-latency seams.
"""

import numpy as np
import ml_dtypes

import concourse.bass as bass
import concourse.tile as tile
import concourse.mybir as mybir
from concourse import bacc
from concourse.bass_utils import run_bass_kernel_spmd

BF16 = mybir.dt.bfloat16
FP8 = mybir.dt.float8e4
F32 = mybir.dt.float32

B, C, H, W = 32, 256, 56, 56
NCORES = 8
BL = B // NCORES          # images per core
KCH = 2                   # 256 = 2 chunks of 128
HP = H + 2                # padded plane side (58)
HW = H * W                # 3136
PL = HP * HP              # 3364

_cache = {}


def _build():
    nc = bacc.Bacc("TRN2", target_bir_lowering=False, debug=False)

    xin_d = nc.dram_tensor("xin", [BL, KCH, 128, PL], FP8, kind="ExternalInput").ap()
    xout_d = nc.dram_tensor("xout", [BL, 128, KCH, HW], BF16, kind="ExternalInput").ap()
    w1_d = nc.dram_tensor("w1t", [128, KCH, 9, 16], FP8, kind="ExternalInput").ap()
    w2_d = nc.dram_tensor("w2t", [128, 2], BF16, kind="ExternalInput").ap()
    out_d = nc.dram_tensor("out", [BL, 128, KCH, HW], BF16, kind="ExternalOutput").ap()

    with tile.TileContext(nc) as tc:
        with (
            tc.tile_pool(name="const", bufs=1) as constp,
            tc.tile_pool(name="xpad", bufs=4) as xpadp,
            tc.tile_pool(name="hcol", bufs=2) as hcolp,
            tc.tile_pool(name="at", bufs=2) as atp,
            tc.tile_pool(name="a9", bufs=2) as a9p,
            tc.tile_pool(name="ab", bufs=2) as abp,
            tc.tile_pool(name="xo", bufs=4) as xop,
            tc.tile_pool(name="ot", bufs=2) as otp,
            tc.tile_pool(name="ps_h", bufs=2, space="PSUM") as ps_h,
            tc.tile_pool(name="ps_a", bufs=2, space="PSUM") as ps_a,
            tc.tile_pool(name="ps_b", bufs=3, space="PSUM") as ps_b,
            tc.tile_pool(name="ps_w", bufs=1, space="PSUM") as ps_w,
        ):
            w1sb = constp.tile([128, KCH, 9, 16], FP8)
            w2sb = constp.tile([128, 2], BF16)
            ones9 = constp.tile([128, 128], BF16)
            wl = constp.tile([128, 128], FP8)
            wr = constp.tile([128, 512], FP8)

            xpads = [
                xpadp.tile([128, KCH, HP, HP], FP8, name="xpad")
                for _ in range(BL)
            ]
            xos = [xop.tile([128, KCH, HW], BF16, name="xo") for _ in range(BL)]
            hcols = [
                hcolp.tile([128, HP, HP], BF16, name="hcol") for _ in range(BL)
            ]
            ats = [atp.tile([128, 2, 7, HP], BF16, name="at") for _ in range(BL)]
            a9s = [a9p.tile([128, HP, HP], BF16, name="a9") for _ in range(BL)]
            MID = 30 * HP

            def load_xin(img, split):
                xpf = xpads[img].rearrange("p k r w -> p k (r w)")
                if split:
                    for k in range(KCH):
                        nc.sync.dma_start(xpf[:, k, 0:MID], xin_d[img, k, :, 0:MID])
                    for k in range(KCH):
                        nc.sync.dma_start(xpf[:, k, MID:PL], xin_d[img, k, :, MID:PL])
                else:
                    for k in range(KCH):
                        nc.sync.dma_start(xpf[:, k, :], xin_d[img, k, :, :])

            def load_xo(img):
                nc.sync.dma_start(xos[img][:], xout_d[img])

            # ---- head loads ----
            xpf0 = xpads[0].rearrange("p k r w -> p k (r w)")
            nc.sync.dma_start(xpf0[:, 0, 0:MID], xin_d[0, 0, :, 0:MID])
            nc.sync.dma_start(w1sb[:], w1_d[:])
            nc.sync.dma_start(w2sb[:], w2_d[:])
            nc.sync.dma_start(xpf0[:, 1, 0:MID], xin_d[0, 1, :, 0:MID])
            nc.sync.dma_start(xpf0[:, 0, MID:PL], xin_d[0, 0, :, MID:PL])
            nc.sync.dma_start(xpf0[:, 1, MID:PL], xin_d[0, 1, :, MID:PL])
            load_xin(1, True)
            load_xo(0)

            # ---- constants / warm fodder; zero guard+pad planes early ----
            nc.vector.memset(ones9[:], 0.0)
            nc.vector.memset(ones9[0:9, :], 1.0)
            nc.gpsimd.memset(wl[:], 0.0)
            nc.gpsimd.memset(wr[:], 0.0)
            for img in range(2):
                nc.scalar.memzero(hcols[img][:, :, :])
                nc.vector.memset(a9s[img][:, :, :], 0.0)
                nc.vector.memset(ats[img][:, :, :, 0], 0.0)
                nc.vector.memset(ats[img][:, :, :, 57], 0.0)

            def warm(n):
                for _ in range(n):
                    wp = ps_w.tile([128, 512], F32)
                    nc.tensor.matmul(
                        wp[:], wl[:], wr[:],
                        start=True, stop=True, skip_group_check=True,
                    )

            def emit_hcol(img):
                """4 chained SP-ring copies build taps 1..7 from the plane."""
                hf = hcols[img].rearrange("p r w -> p (r w)")
                nc.sync.dma_start(hf[16:32, 0 : PL - 1], hf[0:16, 1:PL])
                nc.sync.dma_start(hf[32:48, 0 : PL - 2], hf[0:16, 2:PL])
                nc.sync.dma_start(hf[48:96, 0 : PL - 58], hf[0:48, 58:PL])
                nc.sync.dma_start(hf[96:128, 0 : PL - 116], hf[0:32, 116:PL])

            def gen_conv1(img):
                """36 PE rounds; relu evacs into the h plane on round 18/36."""
                xpad = xpads[img]
                h1 = hcols[img]
                for s in range(2):
                    ps = ps_h.tile([128, 7, 56], F32)
                    rnd = 0
                    for k in range(KCH):
                        for t in range(9):
                            dy, dx = t // 3, t % 3
                            for j in range(4):
                                rs = 28 * s + j + dy
                                nc.tensor.matmul(
                                    ps[32 * j : 32 * j + 16],
                                    w1sb[:, k, t, :],
                                    xpad[:, k, rs : rs + 25 : 4, dx : dx + 56],
                                    start=(rnd == 0),
                                    stop=(rnd == 17),
                                    tile_position=(0, 32 * j),
                                    skip_group_check=True,
                                )
                            rnd += 1
                            if rnd == 18:
                                for j in range(4):
                                    r0 = 1 + 28 * s + j
                                    dst = h1[0:16, r0 : r0 + 25 : 4, 1:57]
                                    if j < 2:
                                        nc.scalar.activation(
                                            dst, ps[32 * j : 32 * j + 16],
                                            mybir.ActivationFunctionType.Relu,
                                        )
                                    else:
                                        nc.vector.tensor_scalar_max(
                                            dst, ps[32 * j : 32 * j + 16], 0.0
                                        )
                            yield

            def gen_conv2(img):
                """4 PE rounds (K-folded); sigmoid; a9 scatter build (Q7)."""
                hcol = hcols[img]
                at = ats[img]
                a9 = a9s[img]
                a9f = a9.rearrange("p r w -> p (r w)")
                for s in range(2):
                    ps = ps_a.tile([128, 7, 56], F32)
                    for j in range(4):
                        b = 4 * s + j
                        nc.tensor.matmul(
                            ps[32 * j : 32 * j + 1],
                            w2sb[:, 0:1],
                            hcol[:, 7 * b : 7 * b + 7, 0:56],
                            start=True, stop=False,
                            tile_position=(0, 32 * j),
                            skip_group_check=True,
                        )
                    yield
                    for j in range(4):
                        b = 4 * s + j
                        nc.tensor.matmul(
                            ps[32 * j : 32 * j + 1],
                            w2sb[:, 1:2],
                            hcol[:, 7 * b + 2 : 7 * b + 9, 2:58],
                            start=False, stop=True,
                            tile_position=(0, 32 * j),
                            skip_group_check=True,
                        )
                    nc.scalar.activation(
                        at[:, s, :, 1:57], ps[:],
                        mybir.ActivationFunctionType.Sigmoid,
                    )
                    for c in range(3):
                        st = (1 + 28 * s) * HP + (1 - c)
                        nc.gpsimd.dma_start(
                            a9f[3 + c : 4 + c, st : st + 1624],
                            at[0:128:32, s],
                        )
                    if s == 1:
                        nc.gpsimd.dma_start(
                            a9f[0:3, HP : 57 * HP], a9f[3:6, 0 : 56 * HP]
                        )
                        nc.gpsimd.dma_start(
                            a9f[6:9, HP : 57 * HP], a9f[3:6, 2 * HP : PL]
                        )
                    yield

            def gen_box(img):
                """7 blocks of 8 rows; evacs split ACT/DVE; bf16 muls."""
                a9 = a9s[img]
                xo = xos[img]
                ab = abp.tile([128, 56, 56], BF16)
                abf = ab.rearrange("p r w -> p (r w)")
                ot = otp.tile([128, KCH, HW], BF16)

                def halfdone(h):
                    s0, s1 = (0, 1792) if h == 0 else (1792, HW)
                    for k in range(KCH):
                        nc.vector.tensor_mul(
                            ot[:, k, s0:s1], xo[:, k, s0:s1], abf[:, s0:s1]
                        )
                    nc.scalar.dma_start(
                        out_d[img, :, :, s0:s1], ot[:, :, s0:s1]
                    )

                for R in range(7):
                    psb = ps_b.tile([128, 8, 56], F32)
                    nc.tensor.matmul(
                        psb[:], ones9[:],
                        a9[:, 1 + 8 * R : 9 + 8 * R, 1:57],
                        start=True, stop=True,
                    )
                    dst = ab[:, 8 * R : 8 * R + 8, :]
                    if R % 2 == 0:
                        nc.scalar.activation(
                            dst, psb[:], mybir.ActivationFunctionType.Copy
                        )
                    else:
                        nc.vector.tensor_copy(dst, psb[:])
                    if R == 3:
                        halfdone(0)
                    yield
                halfdone(1)          # pulled as an 8th next()
                yield

            def run(gen, n):
                for _ in range(n):
                    next(gen, None)

            c1 = [gen_conv1(i) for i in range(BL)]
            c2 = [gen_conv2(i) for i in range(BL)]
            bx = [gen_box(i) for i in range(BL)]

            def block(i):
                # c1 r0-23 solo, conv2(i-1) at r24-27 (1:1), box(i-2) at
                # r28-34 (1:1), last c1 round.  conv2 sits ~5-6.5us into
                # the block (hcol(i-1) ready at +2.5); box(i-2)'s a9 was
                # ready since ~mid-previous-block.
                if i >= 3:
                    run(bx[i - 3], 1)    # deferred h1 muls + store
                run(c1[i], 24)
                for _ in range(4):
                    run(c2[i - 1], 1)
                    run(c1[i], 2)
                for _ in range(4):
                    run(bx[i - 2], 1)
                    run(c1[i], 1)
                run(bx[i - 2], 3)

            warm(3)
            run(c1[0], 36)
            emit_hcol(0)
            load_xin(2, False)
            run(c1[1], 24)
            for _ in range(4):
                run(c2[0], 1)
                run(c1[1], 2)
            run(c1[1], 4)
            emit_hcol(1)
            load_xo(1)
            load_xin(3, False)
            block(2)
            emit_hcol(2)
            load_xo(2)
            block(3)
            emit_hcol(3)
            load_xo(3)
            # tail: finish box(1), conv2(3), box(2), box(3)
            run(bx[1], 1)
            warm(5)
            run(c2[3], 2)
            run(bx[2], 3)
            run(c2[3], 2)
            run(bx[2], 4)
            warm(10)
            run(bx[3], 7)
            run(bx[2], 1)
            run(bx[3], 1)

    nc.compile()
    return nc


def _prep_shards(x_in, x_out, w1, w2):
    bf16 = ml_dtypes.bfloat16
    fp8 = ml_dtypes.float8_e4m3
    # w1t[c, k, t, m] = w1[m, 128k + c, dy, dx],  t = 3*dy + dx
    w1t = np.ascontiguousarray(
        w1.reshape(16, KCH, 128, 9).transpose(2, 1, 3, 0)
    ).astype(fp8)
    # w2t col 0: K-folded taps 0..7 -> w2t[16t + m, 0] = w2[0, m, dy, dx]
    # w2t col 1: tap 8 at partitions 0..15, zero elsewhere
    w2t = np.zeros((128, 2), dtype=bf16)
    w2r = w2[0].reshape(16, 9)
    for t in range(8):
        w2t[16 * t : 16 * t + 16, 0] = w2r[:, t].astype(bf16)
    w2t[0:16, 1] = w2r[:, 8].astype(bf16)
    xi = np.zeros((NCORES, BL, KCH, 128, HP, HP), dtype=fp8)
    xi[..., 1 : 1 + H, 1 : 1 + W] = (
        x_in.reshape(NCORES, BL, KCH, 128, H, W).astype(fp8)
    )
    xi = xi.reshape(NCORES, BL, KCH, 128, PL)
    # xout[img, c_partition, k, hw]
    xo = np.ascontiguousarray(
        x_out.reshape(NCORES, BL, KCH, 128, HW).transpose(0, 1, 3, 2, 4)
    ).astype(bf16)
    return [
        {
            "xin": np.ascontiguousarray(xi[i]),
            "xout": xo[i],
            "w1t": w1t,
            "w2t": w2t,
        }
        for i in range(NCORES)
    ]


def _run(in_maps, trace=False):
    if "nc" not in _cache:
        _cache["nc"] = _build()
    return run_bass_kernel_spmd(
        _cache["nc"], in_maps, core_ids=list(range(NCORES)), trace=trace
    )


def kernel(x_in, x_out, w1, w2, _trace=False):
    in_maps = _prep_shards(
        np.asarray(x_in, dtype=np.float32),
        np.asarray(x_out, dtype=np.float32),
        np.asarray(w1, dtype=np.float32),
        np.asarray(w2, dtype=np.float32),
    )
    res = _run(in_maps, trace=_trace)
    # out[img, c_partition, k, hw] bf16 -> [B, C, H, W] fp32
    out = np.stack([res.results[i]["out"] for i in range(NCORES)])
    kernel.last_exec_time_ns = res.exec_time_ns
    out = out.astype(np.float32).transpose(0, 1, 3, 2, 4)
    return out.reshape(B, C, H, W)
